# revision 1
# baseline (speedup 1.0000x reference)
"""TRN2 Bass kernel for nn_MultiHeadAttention (B=4, S=2048, D=1024, H=16).

v2: host pre-transposes/casts all inputs to fp16 (no PE transposes, no mask
preprocessing on device); per-core program (SPMD over 8 cores): core c handles
batch b=c//2 and query-half qh=c%2.  No collectives.

Device layout (all activations transposed [d, seq], fp16 matmul operands,
f32 PSUM):
  QT [128, 8m x 1024q]  <- Wq^T-chunk @ xqT   (weights stationary)
  KT [128, 8m x 2048k]  <- same for k
  VA [128, 16kst x (16h x 65)] fp16: V-proj in [k, dout] orientation + ones col
  per head h (m=h//2, poff=(h%2)*64):
    per kc (16): sT[128k, 1024q] = KT_h^T @ QT_h (PSUM), p = exp(sT/8) (Act),
      p *= maskT[kc] (DVE fp16 2x), o[65, 1024] += VA_aug^T @ p (PSUM, 16 kc)
    recip = 1/o[64]; recipB = partition_broadcast (Pool);
    UT[poff:, m*SQ:] = o[0:64] * recipB (DVE, fp16)
  out = UT^T @ Wo per 128-q tile (PSUM f32 -> copy -> DMA).
"""
import sys
sys.path.insert(0, "/opt/trn_rl_repo")

import numpy as np
import concourse.bass as bass
import concourse.mybir as mybir
import concourse.tile as tile

F32 = mybir.dt.float32
F16 = mybir.dt.float16
F32R = mybir.dt.float32r
Exp = mybir.ActivationFunctionType.Exp
HD = 64  # head dim (fixed)


def split_ctrl_multiwaits(nc):
    """walrus here rejects >1 sync-wait per instruction; move extras onto
    single-wait NoOps inserted before the instruction on the same engine."""
    n_fixed = 0
    for f in nc.m.functions:
        for bb in f.blocks:
            insts = bb.instructions
            i = 0
            while i < len(insts):
                ins = insts[i]
                si = ins.sync_info
                if si is not None and len(si.on_wait) > 1:
                    waits = list(si.on_wait)
                    si.on_wait = waits[-1:]
                    for j, w in enumerate(waits[:-1]):
                        nop = mybir.InstNoOp(name=f"{ins.name}-ws{j}", ins=[], outs=[])
                        nop.engine = ins.engine
                        nsi = nop.sync_info
                        if nsi is None:
                            nop.sync_info = mybir.SyncInfo(on_wait=[w], on_update=[])
                        else:
                            nsi.on_wait = [w]
                        insts.insert(i, nop)
                        i += 1
                    n_fixed += 1
                i += 1
    return n_fixed


def build_mha(S, D, H, SQ, attn_dt=None, phases=None, repeat=1):
    assert D == H * HD
    ADT = F16 if attn_dt is None else attn_dt
    DC = D // 128           # d-chunks (8)
    NQ = SQ // 128          # q 128-tiles (8)
    NKC = S // 128          # k 128-chunks (16)
    VW = HD + 1             # V_aug columns per head (65)
    HVW = H * VW            # VA columns per k-chunk (1040)

    nc = bass.Bass("TRN2", target_bir_lowering=False, debug=False, num_devices=8)
    xqT = nc.dram_tensor("xqT", [D, SQ], F16, kind="ExternalInput").ap()
    ktd = nc.dram_tensor("ktd", [D, S], F16, kind="ExternalInput").ap()
    vtd = nc.dram_tensor("vtd", [D, S], F16, kind="ExternalInput").ap()
    mkT = nc.dram_tensor("mkT", [S, SQ], F16, kind="ExternalInput").ap()
    wq = nc.dram_tensor("wq", [D, D], F16, kind="ExternalInput").ap()
    wk = nc.dram_tensor("wk", [D, D], F16, kind="ExternalInput").ap()
    wv = nc.dram_tensor("wv", [D, D], F16, kind="ExternalInput").ap()
    wo = nc.dram_tensor("wo", [D, D], F16, kind="ExternalInput").ap()
    out = nc.dram_tensor("out", [SQ, D], F32, kind="ExternalOutput").ap()

    with tile.TileContext(nc) as tc:
        with tc.tile_pool(name="persist", bufs=1) as persist:
          for _rep in range(repeat):
            QT = persist.tile([128, DC * SQ], F16, tag="QT")
            KT = persist.tile([128, DC * S], F16, tag="KT")
            VA = persist.tile([128, NKC * HVW], ADT, tag="VA")
            maskT = persist.tile([128, NKC * SQ], ADT, tag="maskT")
            UT = QT  # rows 0:64 of chunk m are dead (head 2m scored) when written

            # ---------------- Phase A: projections ----------------
            # ones columns of VA (overwritten everywhere except col 64 slots)
            nc.gpsimd.memset(VA[:], 1.0)

            with (
                tc.tile_pool(name="phaqk", bufs=1) as pha,
                tc.tile_pool(name="ppjqk", bufs=4, space="PSUM") as ppj,
            ):
                # --- Q and K projections (one scope: kt DMAs overlap Q proj) ---
                xq_sb = pha.tile([128, DC * SQ], F16, tag="xq_sb")
                wq_sb = pha.tile([128, DC * D], F16, tag="wq_sb")
                kt_sb = pha.tile([128, DC * S], F16, tag="kt_sb")
                wk_sb = pha.tile([128, DC * D], F16, tag="wk_sb")
                for dc in range(DC):
                    nc.sync.dma_start(
                        xq_sb[:, dc * SQ:(dc + 1) * SQ],
                        xqT[dc * 128:(dc + 1) * 128, :])
                    nc.sync.dma_start(
                        wq_sb[:, dc * D:(dc + 1) * D],
                        wq[dc * 128:(dc + 1) * 128, :])
                for dc in range(DC):
                    nc.sync.dma_start(
                        kt_sb[:, dc * S:(dc + 1) * S],
                        ktd[dc * 128:(dc + 1) * 128, :])
                    nc.sync.dma_start(
                        wk_sb[:, dc * D:(dc + 1) * D],
                        wk[dc * 128:(dc + 1) * 128, :])
                # mask loads queued after the projection inputs: not needed
                # until attention, must not delay the first Q-proj matmul
                for kc in range(NKC):
                    nc.sync.dma_start(
                        maskT[:, kc * SQ:(kc + 1) * SQ],
                        mkT[kc * 128:(kc + 1) * 128, :])
                for m in range(DC):
                    ps = ppj.tile([128, 1024], F32, tag="pj")
                    for dc in range(DC):
                        for qs in range(SQ // 512):
                            nc.tensor.matmul(
                                ps[:, qs * 512:(qs + 1) * 512],
                                wq_sb[:, dc * D + m * 128: dc * D + m * 128 + 128],
                                xq_sb[:, dc * SQ + qs * 512: dc * SQ + qs * 512 + 512],
                                start=(dc == 0), stop=(dc == DC - 1))
                    nc.scalar.copy(QT[:, m * SQ:(m + 1) * SQ], ps[:, :SQ])
                for m in range(DC):
                    for ks in range(S // 1024):
                        ps = ppj.tile([128, 1024], F32, tag="pj")
                        for dc in range(DC):
                            for k5 in range(2):
                                nc.tensor.matmul(
                                    ps[:, k5 * 512:(k5 + 1) * 512],
                                    wk_sb[:, dc * D + m * 128: dc * D + m * 128 + 128],
                                    kt_sb[:, dc * S + ks * 1024 + k5 * 512:
                                          dc * S + ks * 1024 + k5 * 512 + 512],
                                    start=(dc == 0), stop=(dc == DC - 1))
                        nc.scalar.copy(
                            KT[:, m * S + ks * 1024: m * S + (ks + 1) * 1024], ps[:])

            # --- V projection (output orientation [k, dout]) ---
            with (
                tc.tile_pool(name="phav", bufs=1) as pha,
                tc.tile_pool(name="ppjv", bufs=4, space="PSUM") as ppj,
            ):
                vt_sb = pha.tile([128, DC * S], F16, tag="vt_sb")
                wv_sb = pha.tile([128, DC * D], F16, tag="wv_sb")
                for dc in range(DC):
                    nc.sync.dma_start(
                        vt_sb[:, dc * S:(dc + 1) * S],
                        vtd[dc * 128:(dc + 1) * 128, :])
                    nc.sync.dma_start(
                        wv_sb[:, dc * D:(dc + 1) * D],
                        wv[dc * 128:(dc + 1) * 128, :])
                for kst in range(NKC):
                    ps = ppj.tile([128, 1024], F32, tag="pj")
                    for dc in range(DC):
                        for j in range(2):
                            nc.tensor.matmul(
                                ps[:, j * 512:(j + 1) * 512],
                                vt_sb[:, dc * S + kst * 128: dc * S + (kst + 1) * 128],
                                wv_sb[:, dc * D + j * 512: dc * D + j * 512 + 512],
                                start=(dc == 0), stop=(dc == DC - 1))
                    # scatter 16 heads x 64 into VA (stride VW leaves ones col)
                    dst = VA[:, kst * HVW:(kst + 1) * HVW].rearrange(
                        "p (h x) -> p h x", x=VW)[:, :, 0:HD]
                    nc.scalar.copy(
                        dst, ps.rearrange("p (h x) -> p h x", x=HD))

            # ---------------- Phase B: attention per head ----------------
            with (
                tc.tile_pool(name="phb", bufs=1) as phb,
                tc.tile_pool(name="pp", bufs=4) as pp,
                tc.tile_pool(name="pS", bufs=2, space="PSUM") as pS,
                tc.tile_pool(name="pO", bufs=1, space="PSUM") as pO,
            ):
                ones64 = phb.tile([1, 64], F16, tag="ones64")
                nc.gpsimd.memset(ones64[:], 1.0)
                wo_sb = phb.tile([128, DC * D], F16, tag="wo_sb")
                for dc in range(DC):
                    nc.sync.dma_start(
                        wo_sb[:, dc * D:(dc + 1) * D],
                        wo[dc * 128:(dc + 1) * 128, :])
                for h in range(H):
                    m, poff = h // 2, (h % 2) * 64
                    o = pO.tile([128, SQ], F32, tag=f"o{h % 2}", name=f"o{h}")
                    for kc in range(NKC):
                        sT = pS.tile([128, SQ], F32, tag="sT")
                        for qs in range(SQ // 512):
                            nc.tensor.matmul(
                                sT[:, qs * 512:(qs + 1) * 512],
                                KT[poff:poff + HD,
                                   m * S + kc * 128: m * S + (kc + 1) * 128],
                                QT[poff:poff + HD,
                                   m * SQ + qs * 512: m * SQ + qs * 512 + 512],
                                start=True, stop=True)
                        p = pp.tile([128, SQ], ADT, tag="p")
                        nc.scalar.activation(p[:], sT[:], Exp, scale=0.125)
                        nc.vector.tensor_mul(
                            p[:], p[:], maskT[:, kc * SQ:(kc + 1) * SQ])
                        for qs in range(SQ // 512):
                            nc.tensor.matmul(
                                o[0:VW, qs * 512:(qs + 1) * 512],
                                VA[:, kc * HVW + h * VW: kc * HVW + (h + 1) * VW],
                                p[:, qs * 512:(qs + 1) * 512],
                                start=(kc == 0), stop=(kc == NKC - 1))
                    recip = phb.tile([1, SQ], F32, tag="recip", bufs=2)
                    nc.vector.reciprocal(recip[:], o[HD:HD + 1, :])
                    recipR = phb.tile([1, SQ], F16, tag="recip16", bufs=2)
                    nc.vector.tensor_copy(recipR[:], recip[:])
                    # broadcast recip to partitions 64..127 of o (sums row is
                    # dead once recip is computed) via a ones-column matmul
                    for qs in range(SQ // 512):
                        nc.tensor.matmul(
                            o[64:128, qs * 512:(qs + 1) * 512],
                            ones64[:], recipR[:, qs * 512:(qs + 1) * 512],
                            start=True, stop=True)
                    nc.vector.tensor_copy(
                        UT[poff:poff + 64, m * SQ:(m + 1) * SQ], o[0:HD, :])
                    nc.vector.tensor_mul(
                        UT[poff:poff + 64, m * SQ:(m + 1) * SQ],
                        UT[poff:poff + 64, m * SQ:(m + 1) * SQ], o[64:128, :])

                # ---- Phase C: output projection (same scope, pO psum) ----
                for qt in range(NQ):
                    ps = pO.tile([128, SQ], F32, tag=f"o{qt % 2}", name=f"oc{qt}")
                    for dc in range(DC):
                        for j in range(2):
                            nc.tensor.matmul(
                                ps[:, j * 512:(j + 1) * 512],
                                UT[:, dc * SQ + qt * 128: dc * SQ + (qt + 1) * 128],
                                wo_sb[:, dc * D + j * 512: dc * D + j * 512 + 512],
                                start=(dc == 0), stop=(dc == DC - 1))
                    ot = phb.tile([128, D], F32, tag="ot", bufs=2)
                    nc.scalar.copy(ot[:], ps[:, :D])
                    nc.sync.dma_start(out[qt * 128:(qt + 1) * 128, :], ot[:])

    return nc


"""Shared runner: execute a Bass program on the 8 axon-tunneled NeuronCores
via bass2jax, with support for repeated calls (steady-state wall timing)."""
import time
import jax
from jax.sharding import Mesh, PartitionSpec
from jax.experimental.shard_map import shard_map

from concourse.bass2jax import _bass_exec_p, install_neuronx_cc_hook, partition_id_tensor


class SpmdRunner:
    def __init__(self, nc, n_cores):
        install_neuronx_cc_hook()
        self.nc = nc
        self.n_cores = n_cores
        partition_name = nc.partition_id_tensor.name if nc.partition_id_tensor else None
        in_names, out_names, out_avals = [], [], []
        for alloc in nc.m.functions[0].allocations:
            if not isinstance(alloc, mybir.MemoryLocationSet):
                continue
            name = alloc.memorylocations[0].name
            if alloc.kind == "ExternalInput":
                if name != partition_name:
                    in_names.append(name)
            elif alloc.kind == "ExternalOutput":
                out_names.append(name)
                shape = tuple(alloc.tensor_shape)
                dtype = mybir.dt.np(alloc.dtype)
                out_avals.append(jax.core.ShapedArray(shape, dtype))
        self.in_names, self.out_names, self.out_avals = in_names, out_names, out_avals
        n_params = len(in_names)
        all_names = list(in_names) + list(out_names)
        if partition_name is not None:
            all_names.append(partition_name)

        def _body(*args):
            operands = list(args)
            if partition_name is not None:
                operands.append(partition_id_tensor())
            outs = _bass_exec_p.bind(
                *operands,
                out_avals=tuple(out_avals),
                in_names=tuple(all_names),
                out_names=tuple(out_names),
                lowering_input_output_aliases=(),
                sim_require_finite=True,
                sim_require_nnan=True,
                nc=nc,
            )
            return tuple(outs)

        devices = jax.devices()[:n_cores]
        self.mesh = Mesh(np.asarray(devices), ("core",))
        in_specs = (PartitionSpec("core"),) * (n_params + len(out_names))
        out_specs = (PartitionSpec("core"),) * len(out_names)
        self.fn = jax.jit(
            shard_map(_body, mesh=self.mesh, in_specs=in_specs,
                      out_specs=out_specs, check_rep=False),
            keep_unused=True,
        )
        self.n_params = n_params

    def stage(self, in_maps):
        n = self.n_cores
        assert len(in_maps) == n
        concat_in = [
            np.concatenate([np.asarray(in_maps[c][name]) for c in range(n)], axis=0)
            for name in self.in_names
        ]
        concat_zeros = [
            np.zeros((n * a.shape[0], *a.shape[1:]), a.dtype) for a in self.out_avals
        ]
        self.args = [jax.device_put(a) for a in concat_in + concat_zeros]
        return self

    def run(self):
        outs = self.fn(*self.args)
        jax.block_until_ready(outs)
        return outs

    def results(self, outs):
        n = self.n_cores
        return [
            {
                name: np.asarray(outs[i]).reshape(n, *self.out_avals[i].shape)[c]
                for i, name in enumerate(self.out_names)
            }
            for c in range(n)
        ]

    def time_runs(self, iters=10, warmup=2):
        for _ in range(warmup):
            self.run()
        ts = []
        for _ in range(iters):
            t0 = time.perf_counter()
            self.run()
            ts.append(time.perf_counter() - t0)
        return min(ts), float(np.median(ts)), max(ts)

    def _run_batch(self, m):
        outs = None
        t0 = time.perf_counter()
        for _ in range(m):
            outs = self.fn(*self.args)
        jax.block_until_ready(outs)
        return time.perf_counter() - t0

    def time_async(self, m1=4, m2=36, reps=6):
        self.run()
        w1 = min(self._run_batch(m1) for _ in range(reps))
        w2 = min(self._run_batch(m2) for _ in range(reps))
        return (w2 - w1) / (m2 - m1), w1, w2


# ----------------------------------------------------------------------------
# Host-side entry: shard full inputs over the 8 NeuronCores, run, gather.
# ----------------------------------------------------------------------------
B, S, D, H = 4, 2048, 1024, 16
SQ = S // 2
NCORES = 8

_runner_cache = []


def _get_runner():
    if not _runner_cache:
        nc = build_mha(S, D, H, SQ, attn_dt=mybir.dt.float16)
        split_ctrl_multiwaits(nc)
        _runner_cache.append(SpmdRunner(nc, NCORES))
    return _runner_cache[0]


def _make_in_maps(q, k, v, mask, Wq, Wk, Wv, Wo):
    f16 = np.float16
    wq16 = np.ascontiguousarray(Wq, dtype=f16)
    wk16 = np.ascontiguousarray(Wk, dtype=f16)
    wv16 = np.ascontiguousarray(Wv, dtype=f16)
    wo16 = np.ascontiguousarray(Wo, dtype=f16)
    in_maps = []
    for c in range(NCORES):
        b, qh = c // 2, c % 2
        in_maps.append({
            "xqT": np.ascontiguousarray(q[b, qh * SQ:(qh + 1) * SQ].T, dtype=f16),
            "ktd": np.ascontiguousarray(k[b].T, dtype=f16),
            "vtd": np.ascontiguousarray(v[b].T, dtype=f16),
            "mkT": np.ascontiguousarray(mask[b, qh * SQ:(qh + 1) * SQ].T, dtype=f16),
            "wq": wq16, "wk": wk16, "wv": wv16, "wo": wo16,
        })
    return in_maps


def kernel(q, k, v, mask, Wq, Wk, Wv, Wo):
    q = np.asarray(q, np.float32)
    k = np.asarray(k, np.float32)
    v = np.asarray(v, np.float32)
    mask = np.asarray(mask, np.int32)
    Wq, Wk, Wv, Wo = (np.asarray(a, np.float32) for a in (Wq, Wk, Wv, Wo))
    r = _get_runner()
    r.stage(_make_in_maps(q, k, v, mask, Wq, Wk, Wv, Wo))
    res = r.results(r.run())
    out = np.empty((B, S, D), np.float32)
    for c in range(NCORES):
        b, qh = c // 2, c % 2
        out[b, qh * SQ:(qh + 1) * SQ] = res[c]["out"]
    return out



# revision 37
# speedup vs baseline: 1.0262x; 1.0262x over previous
"""TRN2 Bass kernel for nn_MultiHeadAttention (B=4, S=2048, D=1024, H=16).

v3: per-core program (SPMD over 8 cores): core c handles batch b=c//2 and
query-half qh=c%2 (SQ=1024 queries, full S=2048 keys).  No collectives.

Key structure (fp16 matmul operands, f32 PSUM):
  - Projections Q/K/V with weights stationary; PSUM evictions on Pool/DVE so
    the Act engine runs ONLY the 256 exp instructions (the hard floor).
  - Attention per (head h, key-chunk kc):
      sT[128k, 1024q] = KT_h^T @ QT_h   (PSUM; optional fp8-DoubleRow)
      p = exp(sT/8) (Act); p *= maskT[kc] (DVE fp16 2x_1p)
      PV FLIPPED: p-chunk [128k, 128q] is the STATIONARY operand, moving is
      VA_aug [128k, 65] (V-proj + ones column), so the PE streams only 65
      columns per (h,kc,qc) into o[128q, 65-slices @128] - half the model
      cost of the [65, 1024q] orientation.  Denominator rides in column 64.
  - Normalize on DVE: reciprocal of den col + per-partition tensor_scalar.
  - U [q, d] -> UT [d, q] chunks via DMA-engine transpose (frees PE/DVE);
    out = UT^T @ Wo per 128-q tile.
  - Emission interleaves projections/V-chunks into the head loop (filler
    sprinkler) and defers PV by LAG kc-slots so engines stay fed.
"""
import sys
sys.path.insert(0, "/opt/trn_rl_repo")

import numpy as np
import concourse.bass as bass
import concourse.mybir as mybir
import concourse.tile as tile

F32 = mybir.dt.float32
F16 = mybir.dt.float16
F8 = mybir.dt.float8e4
Exp = mybir.ActivationFunctionType.Exp
DR = mybir.MatmulPerfMode.DoubleRow
HD = 64  # head dim (fixed)


def split_ctrl_multiwaits(nc):
    """walrus here rejects >1 sync-wait per instruction; move extras onto
    single-wait NoOps inserted before the instruction on the same engine."""
    n_fixed = 0
    for f in nc.m.functions:
        for bb in f.blocks:
            insts = bb.instructions
            i = 0
            while i < len(insts):
                ins = insts[i]
                si = ins.sync_info
                if si is not None and len(si.on_wait) > 1:
                    waits = list(si.on_wait)
                    si.on_wait = waits[-1:]
                    for j, w in enumerate(waits[:-1]):
                        nop = mybir.InstNoOp(name=f"{ins.name}-ws{j}", ins=[], outs=[])
                        nop.engine = ins.engine
                        nsi = nop.sync_info
                        if nsi is None:
                            nop.sync_info = mybir.SyncInfo(on_wait=[w], on_update=[])
                        else:
                            nsi.on_wait = [w]
                        insts.insert(i, nop)
                        i += 1
                    n_fixed += 1
                i += 1
    return n_fixed


def build_mha(S, D, H, SQ, attn_dt=None, phases=None, repeat=1, qk8=False):
    assert D == H * HD
    ADT = F16 if attn_dt is None else attn_dt
    DC = D // 128            # d-chunks (8)
    NQ = SQ // 128           # q 128-tiles (8)
    NKC = S // 128           # k 128-chunks (16)
    NKS = S // 512           # k 512-col groups (4)
    VW = HD + 1              # V_aug columns per head (65)
    HVW = H * VW             # VA columns per k-chunk (1040)
    LAG = 3                  # PV trails exp by LAG kc-slots

    nc = bass.Bass("TRN2", target_bir_lowering=False, debug=False, num_devices=8)
    xqT = nc.dram_tensor("xqT", [D, SQ], F16, kind="ExternalInput").ap()
    ktd = nc.dram_tensor("ktd", [D, S], F16, kind="ExternalInput").ap()
    vtd = nc.dram_tensor("vtd", [D, S], F16, kind="ExternalInput").ap()
    mkT = nc.dram_tensor("mkT", [S, SQ], F16, kind="ExternalInput").ap()
    wq = nc.dram_tensor("wq", [D, D], F16, kind="ExternalInput").ap()
    wk = nc.dram_tensor("wk", [D, D], F16, kind="ExternalInput").ap()
    wv = nc.dram_tensor("wv", [D, D], F16, kind="ExternalInput").ap()
    wo = nc.dram_tensor("wo", [D, D], F16, kind="ExternalInput").ap()
    out = nc.dram_tensor("out", [SQ, D], F32, kind="ExternalOutput").ap()
    # nonce: the bass2jax HLO hash doesn't cover the embedded program, so a
    # stale NEFF cache entry could silently run an old kernel.  A per-build
    # named input forces a unique module hash.
    import hashlib as _hl
    _tag = _hl.md5(
        f"v3:{S}:{D}:{H}:{SQ}:{repeat}:{qk8}:{attn_dt}:r11".encode()
    ).hexdigest()[:10]
    nc.dram_tensor(f"nz_{_tag}", [1, 1], F32, kind="ExternalInput")

    with tile.TileContext(nc) as tc:
      for _rep in range(repeat):
        with (
            tc.tile_pool(name="persist", bufs=1) as persist,
            tc.tile_pool(name="pqt", bufs=2) as pqt,
            tc.tile_pool(name="pkt", bufs=2) as pkt,
            tc.tile_pool(name="pp", bufs=LAG + 1) as ppool,
            tc.tile_pool(name="puh", bufs=2) as puh,
            tc.tile_pool(name="prc", bufs=2) as prc,
        ):
            maskT = persist.tile([128, NKC * SQ], F16, tag="maskT")
            VA = persist.tile([128, NKC * HVW], F16, tag="VA")
            UTall = persist.tile([128, NQ * DC * 128], F16, tag="UTall")
            nc.gpsimd.memset(VA[:], 1.0)

            # ---------------- state shared across scopes ----------------
            QT, KT = {}, {}
            p_tiles, o_tiles, u_tiles, vt_tiles = {}, {}, {}, {}
            pv_queue = []
            fillers = []

            def q_proj(m, half):
                """one [128, 512] psum chunk of Q-proj for m-chunk."""
                ps = psT.tile([128, SQ], F32, tag="sT", name=f"qp{m}_{half}")[:, 0:512]
                for dc in range(DC):
                    nc.tensor.matmul(
                        ps[:],
                        wq_sb[:, dc * D + m * 128: dc * D + m * 128 + 128],
                        xq_sb[:, dc * SQ + half * 512: dc * SQ + half * 512 + 512],
                        start=(dc == 0), stop=(dc == DC - 1))
                if qk8:
                    t = QT.get(m)
                    if t is None:
                        t = QT[m] = pqt.tile([64, 2 * SQ], F8, tag="QT", name=f"QT{m}")
                    for e in range(2):
                        for s in range(2):
                            eng = nc.vector
                            eng.tensor_copy(
                                t[e * 32:(e + 1) * 32,
                                  s * SQ + half * 512: s * SQ + half * 512 + 512],
                                ps[e * 64 + s * 32: e * 64 + s * 32 + 32, :])
                else:
                    t = QT.get(m)
                    if t is None:
                        t = QT[m] = pqt.tile([128, SQ], F16, tag="QT", name=f"QT{m}")
                    nc.vector.tensor_copy(
                        t[:, half * 512:(half + 1) * 512], ps[:])

            def k_proj(m, ks):
                """one [128, 512] psum chunk of K-proj for (m, ks)."""
                ps = psT.tile([128, SQ], F32, tag="sT", name=f"kp{m}_{ks}")[:, 0:512]
                for dc in range(DC):
                    nc.tensor.matmul(
                        ps[:],
                        wk_sb[:, dc * D + m * 128: dc * D + m * 128 + 128],
                        kt_sb[:, dc * S + ks * 512: dc * S + ks * 512 + 512],
                        start=(dc == 0), stop=(dc == DC - 1))
                if qk8:
                    t = KT.get(m)
                    if t is None:
                        t = KT[m] = pkt.tile([64, 2 * S], F8, tag="KT", name=f"KT{m}")
                    for e in range(2):
                        for s in range(2):
                            eng = nc.vector
                            eng.tensor_copy(
                                t[e * 32:(e + 1) * 32,
                                  s * S + ks * 512: s * S + ks * 512 + 512],
                                ps[e * 64 + s * 32: e * 64 + s * 32 + 32, :])
                else:
                    t = KT.get(m)
                    if t is None:
                        t = KT[m] = pkt.tile([128, S], F16, tag="KT", name=f"KT{m}")
                    nc.vector.tensor_copy(
                        t[:, ks * 512:(ks + 1) * 512], ps[:])

            v_half_queue = [(kst, hf) for kst in range(NKC) for hf in (0, 1)]

            def vt_load(kst):
                # gpsimd (SWDGE) queue: doesn't sit behind the big SP-queue
                # input loads.  One 3D-AP DMA per kst chunk.
                t = pvt.tile([128, DC * 128], F16, tag="vt", name=f"vt{kst}")
                vt_tiles[kst] = t
                src = vtd[:, kst * 128:(kst + 1) * 128].rearrange(
                    "(dc p) c -> p dc c", p=128)
                nc.gpsimd.dma_start(
                    t.rearrange("p (dc c) -> p dc c", c=128), src)

            def v_proj(kst, half):
                """V-proj chunk: out [128 k(kst), 512 dout] -> VA 65-scatter."""
                ps = psT.tile([128, SQ], F32, tag="sT",
                              name=f"vp{kst}_{half}")[:, 0:512]
                for dc in range(DC):
                    nc.tensor.matmul(
                        ps[:],
                        vt_tiles[kst][:, dc * 128:(dc + 1) * 128],
                        wv_sb[:, dc * D + half * 512: dc * D + half * 512 + 512],
                        start=(dc == 0), stop=(dc == DC - 1))
                dst = VA[:, kst * HVW + half * 8 * VW:
                         kst * HVW + half * 8 * VW + 8 * VW].rearrange(
                    "p (h x) -> p h x", x=VW)[:, :, 0:HD]
                nc.vector.tensor_copy(
                    dst, ps.rearrange("p (h x) -> p h x", x=HD))

            def emit_v_chunk():
                if not v_half_queue:
                    return False
                kst, half = v_half_queue.pop(0)
                if half == 0:
                    for la in (0, 1):
                        if kst + la < NKC and kst + la not in vt_tiles:
                            vt_load(kst + la)
                v_proj(kst, half)
                if half == 1:
                    del vt_tiles[kst]
                return True

            def emit_qk_exp(h, kc):
                m, e = h // 2, h % 2
                sT = psT.tile([128, SQ], F32, tag="sT", name=f"s{h}_{kc}")
                if qk8:
                    kt8 = KT[m][e * 32:(e + 1) * 32, :].rearrange(
                        "p (s k) -> p s k", s=2)
                    qt8 = QT[m][e * 32:(e + 1) * 32, :].rearrange(
                        "p (s q) -> p s q", s=2)
                    for qs in range(SQ // 512):
                        nc.tensor.matmul(
                            sT[:, qs * 512:(qs + 1) * 512],
                            kt8[:, :, kc * 128:(kc + 1) * 128],
                            qt8[:, :, qs * 512:(qs + 1) * 512],
                            start=True, stop=True, perf_mode=DR)
                else:
                    poff = e * 64
                    for qs in range(SQ // 512):
                        nc.tensor.matmul(
                            sT[:, qs * 512:(qs + 1) * 512],
                            KT[m][poff:poff + HD, kc * 128:(kc + 1) * 128],
                            QT[m][poff:poff + HD, qs * 512:(qs + 1) * 512],
                            start=True, stop=True)
                p = ppool.tile([128, SQ], ADT, tag="p", name=f"p{h}_{kc}")
                nc.scalar.activation(p[:], sT[:], Exp, scale=0.125)
                nc.vector.tensor_mul(
                    p[:], p[:], maskT[:, kc * SQ:(kc + 1) * SQ])
                p_tiles[(h, kc)] = p

            def emit_pv(h, kc):
                o = o_tiles.get(h)
                if o is None:
                    o = o_tiles[h] = pO.tile([128, 1024], F32, tag="o",
                                             name=f"o{h}")
                p = p_tiles.pop((h, kc))
                for qc in range(NQ):
                    nc.tensor.matmul(
                        o[:, qc * 128: qc * 128 + VW],
                        p[:, qc * 128:(qc + 1) * 128],
                        VA[:, kc * HVW + h * VW: kc * HVW + (h + 1) * VW],
                        start=(kc == 0 and qc % 4 == 0),
                        stop=(kc == NKC - 1 and qc % 4 == 3),
                        skip_group_check=True)

            def emit_norm(h):
                """recip + normalize head h, write U_hp; free o tile."""
                o = o_tiles.pop(h)
                hp = h // 2
                u = u_tiles.get(hp)
                if u is None:
                    u = u_tiles[hp] = puh.tile([128, NQ * 128], F16, tag="Uhp",
                                               name=f"u{hp}")
                rc = prc.tile([128, NQ], F32, tag="recip", name=f"rc{h}")
                den = o.rearrange("p (qc x) -> p qc x", x=128)[
                    :, :, HD:HD + 1].rearrange("p a b -> p (a b)")
                nc.vector.reciprocal(rc[:], den)
                for qc in range(NQ):
                    nc.vector.tensor_scalar(
                        u[:, qc * 128 + (h % 2) * HD:
                          qc * 128 + (h % 2) * HD + HD],
                        o[:, qc * 128: qc * 128 + HD],
                        rc[:, qc:qc + 1], None, mybir.AluOpType.mult)

            def emit_transposes(hp):
                """U_hp [q, 128d] chunks -> UTall [d, q] via DMA transpose."""
                u = u_tiles.pop(hp)
                for qc in range(NQ):
                    nc.sync.dma_start_transpose(
                        UTall[:, (qc * DC + hp) * 128:(qc * DC + hp) * 128 + 128],
                        u[:, qc * 128:(qc + 1) * 128])

            TRANS_AT = {3: 0, 4: 1, 6: 2, 8: 3, 10: 4, 12: 5, 14: 6}

            def head_slot(h, kc):
                """one kc-slot of the software pipeline.  The NEXT slot's
                QK+exp+mask are emitted before this slot's heavy PE work so
                the Act engine never waits behind V/filler/PV bursts."""
                slot = h * NKC + kc
                if slot == 0:
                    emit_qk_exp(0, 0)
                nslot = slot + 1
                if nslot < H * NKC:
                    emit_qk_exp(nslot // NKC, nslot % NKC)
                pv_queue.append((h, kc))
                if kc == 4 and h in TRANS_AT:
                    emit_transposes(TRANS_AT[h])
                if h < 2:
                    emit_v_chunk()
                    emit_v_chunk()
                # fillers throttled by deadline group (QT/KT pools have only
                # 2 buffers: group m may only start at head 2(m-1))
                due = h // 2 + 1
                npop = 0
                while (fillers and fillers[0][0] <= slot
                       and fillers[0][1] <= due and npop < 2):
                    fillers.pop(0)[2]()
                    npop += 1
                if len(pv_queue) > LAG:
                    emit_pv(*pv_queue.pop(0))
                    nh = pv_queue[0][0]
                    if pv_queue[0][1] == 0 and nh > 0:
                        emit_norm(nh - 1)

            # ---------------- DMA priority order ----------------
            with tc.tile_pool(name="psT", bufs=2, space="PSUM") as psT, \
                 tc.tile_pool(name="pO", bufs=2, space="PSUM") as pO, \
                 tc.tile_pool(name="pin", bufs=1) as pin:
                wq_sb = pin.tile([128, DC * D], F16, tag="wq_sb")
                xq_sb = pin.tile([128, DC * SQ], F16, tag="xq_sb")
                wk_sb = pin.tile([128, DC * D], F16, tag="wk_sb")
                kt_sb = pin.tile([128, DC * S], F16, tag="kt_sb")

                def load_wide(dst, src, w, pieces=2):
                    # few 3D-AP DMAs: [D=dc*128 rows, w cols] -> [128, dc, w];
                    # ~1MB granularity so the shared DMA engines aren't held
                    # hostage by one monolithic transfer.
                    g = DC // pieces
                    for i in range(pieces):
                        nc.sync.dma_start(
                            dst.rearrange("p (dc w) -> p dc w", w=w)[
                                :, i * g:(i + 1) * g, :],
                            src.rearrange("(dc p) w -> p dc w", p=128)[
                                :, i * g:(i + 1) * g, :])

                def load_cols(dst, srcd, c0, c1):
                    # one 3D-AP DMA for columns [c0:c1) of every dc-chunk
                    nc.sync.dma_start(
                        dst.rearrange("p (dc w) -> p dc w", w=D)[:, :, c0:c1],
                        srcd.rearrange("(dc p) w -> p dc w", p=128)[:, :, c0:c1])

                load_cols(wq_sb, wq, 0, 128)
                for dc in range(DC):
                    nc.sync.dma_start(
                        xq_sb[:, dc * SQ:(dc + 1) * SQ],
                        xqT[dc * 128:(dc + 1) * 128, :])
                load_cols(wk_sb, wk, 0, 128)

                def kt_group(ks):
                    for dc in range(DC):
                        nc.sync.dma_start(
                            kt_sb[:, dc * S + ks * 512: dc * S + (ks + 1) * 512],
                            ktd[dc * 128:(dc + 1) * 128, ks * 512:(ks + 1) * 512])

                def mask_group(kcs):
                    for kc in kcs:
                        nc.sync.dma_start(
                            maskT[:, kc * SQ:(kc + 1) * SQ],
                            mkT[kc * 128:(kc + 1) * 128, :])

                kt_group(0)
                mask_group([0, 1])

                # filler order: finish K m0, then per-m Q/K groups
                # (gate_slot, group, fn)
                for ks in range(1, NKS):
                    fillers.append((0, 0, lambda ks=ks: k_proj(0, ks)))
                for m in range(1, DC):
                    fillers.append((0, m, lambda m=m: q_proj(m, 0)))
                    fillers.append((0, m, lambda m=m: q_proj(m, 1)))
                    for ks in range(NKS):
                        fillers.append((0, m, lambda m=m, ks=ks: k_proj(m, ks)))

                with tc.tile_pool(name="pvw", bufs=1) as pvw, \
                     tc.tile_pool(name="pvt", bufs=2) as pvt:
                    wv_sb = pvw.tile([128, DC * D], F16, tag="wv_sb")
                    for dc in range(DC):
                        nc.sync.dma_start(
                            wv_sb[:, dc * D:(dc + 1) * D],
                            wv[dc * 128:(dc + 1) * 128, :])
                    # remaining SP-queue loads in deadline order
                    kt_group(1)
                    mask_group([2, 3])
                    load_cols(wq_sb, wq, 128, 576)
                    load_cols(wk_sb, wk, 128, 576)
                    kt_group(2)
                    mask_group([4, 5])
                    load_cols(wq_sb, wq, 576, 1024)
                    load_cols(wk_sb, wk, 576, 1024)
                    kt_group(3)
                    mask_group(range(6, NKC))

                    # startup: first Q/K chunks so head 0 can run
                    k_proj(0, 0)
                    q_proj(0, 0)
                    q_proj(0, 1)

                    for h in range(3):
                        for kc in range(NKC):
                            head_slot(h, kc)
                    while v_half_queue:
                        emit_v_chunk()

                for h in range(3, H):
                    for kc in range(NKC):
                        head_slot(h, kc)
                # drain
                while fillers:
                    fillers.pop(0)[2]()
                while pv_queue:
                    emit_pv(*pv_queue.pop(0))
                emit_norm(H - 1)
                emit_transposes(H // 2 - 1)


            # ---------------- phase C: output projection ----------------
            with (
                tc.tile_pool(name="phc", bufs=1) as phc,
                tc.tile_pool(name="poc", bufs=2, space="PSUM") as poc,
                tc.tile_pool(name="pot", bufs=2) as pot,
            ):
                wo_sb = phc.tile([128, DC * D], F16, tag="wo_sb")
                nc.sync.dma_start(
                    wo_sb.rearrange("p (dc w) -> p dc w", w=D),
                    wo.rearrange("(dc p) w -> p dc w", p=128))
                for qt in range(NQ):
                    ps = poc.tile([128, D], F32, tag="oc", name=f"oc{qt}")
                    for dc in range(DC):
                        for j in range(2):
                            nc.tensor.matmul(
                                ps[:, j * 512:(j + 1) * 512],
                                UTall[:, (qt * DC + dc) * 128:
                                      (qt * DC + dc) * 128 + 128],
                                wo_sb[:, dc * D + j * 512: dc * D + j * 512 + 512],
                                start=(dc == 0), stop=(dc == DC - 1))
                    ot = pot.tile([128, D], F32, tag="ot", name=f"ot{qt}")
                    nc.vector.tensor_copy(ot[:], ps[:])
                    nc.sync.dma_start(out[qt * 128:(qt + 1) * 128, :], ot[:])

    return nc


"""Shared runner: execute a Bass program on the 8 axon-tunneled NeuronCores
via bass2jax, with support for repeated calls (steady-state wall timing)."""
import time
import jax
from jax.sharding import Mesh, PartitionSpec
from jax.experimental.shard_map import shard_map

from concourse.bass2jax import _bass_exec_p, install_neuronx_cc_hook, partition_id_tensor


class SpmdRunner:
    def __init__(self, nc, n_cores):
        install_neuronx_cc_hook()
        self.nc = nc
        self.n_cores = n_cores
        partition_name = nc.partition_id_tensor.name if nc.partition_id_tensor else None
        in_names, out_names, out_avals = [], [], []
        for alloc in nc.m.functions[0].allocations:
            if not isinstance(alloc, mybir.MemoryLocationSet):
                continue
            name = alloc.memorylocations[0].name
            if alloc.kind == "ExternalInput":
                if name != partition_name:
                    in_names.append(name)
            elif alloc.kind == "ExternalOutput":
                out_names.append(name)
                shape = tuple(alloc.tensor_shape)
                dtype = mybir.dt.np(alloc.dtype)
                out_avals.append(jax.core.ShapedArray(shape, dtype))
        self.in_names, self.out_names, self.out_avals = in_names, out_names, out_avals
        n_params = len(in_names)
        all_names = list(in_names) + list(out_names)
        if partition_name is not None:
            all_names.append(partition_name)

        def _body(*args):
            operands = list(args)
            if partition_name is not None:
                operands.append(partition_id_tensor())
            outs = _bass_exec_p.bind(
                *operands,
                out_avals=tuple(out_avals),
                in_names=tuple(all_names),
                out_names=tuple(out_names),
                lowering_input_output_aliases=(),
                sim_require_finite=True,
                sim_require_nnan=True,
                nc=nc,
            )
            return tuple(outs)

        devices = jax.devices()[:n_cores]
        self.mesh = Mesh(np.asarray(devices), ("core",))
        in_specs = (PartitionSpec("core"),) * (n_params + len(out_names))
        out_specs = (PartitionSpec("core"),) * len(out_names)
        self.fn = jax.jit(
            shard_map(_body, mesh=self.mesh, in_specs=in_specs,
                      out_specs=out_specs, check_rep=False),
            keep_unused=True,
        )
        self.n_params = n_params

    def stage(self, in_maps):
        n = self.n_cores
        assert len(in_maps) == n
        shapes = {}
        for alloc in self.nc.m.functions[0].allocations:
            if isinstance(alloc, mybir.MemoryLocationSet):
                shapes[alloc.memorylocations[0].name] = (
                    tuple(alloc.tensor_shape), mybir.dt.np(alloc.dtype))
        def get(c, name):
            if name in in_maps[c]:
                return np.asarray(in_maps[c][name])
            shape, dt = shapes[name]
            return np.zeros(shape, dt)
        concat_in = [
            np.concatenate([get(c, name) for c in range(n)], axis=0)
            for name in self.in_names
        ]
        concat_zeros = [
            np.zeros((n * a.shape[0], *a.shape[1:]), a.dtype) for a in self.out_avals
        ]
        self.args = [jax.device_put(a) for a in concat_in + concat_zeros]
        return self

    def run(self):
        outs = self.fn(*self.args)
        jax.block_until_ready(outs)
        return outs

    def results(self, outs):
        n = self.n_cores
        return [
            {
                name: np.asarray(outs[i]).reshape(n, *self.out_avals[i].shape)[c]
                for i, name in enumerate(self.out_names)
            }
            for c in range(n)
        ]

    def time_runs(self, iters=10, warmup=2):
        for _ in range(warmup):
            self.run()
        ts = []
        for _ in range(iters):
            t0 = time.perf_counter()
            self.run()
            ts.append(time.perf_counter() - t0)
        return min(ts), float(np.median(ts)), max(ts)

    def _run_batch(self, m):
        outs = None
        t0 = time.perf_counter()
        for _ in range(m):
            outs = self.fn(*self.args)
        jax.block_until_ready(outs)
        return time.perf_counter() - t0

    def time_async(self, m1=4, m2=36, reps=6):
        self.run()
        w1 = min(self._run_batch(m1) for _ in range(reps))
        w2 = min(self._run_batch(m2) for _ in range(reps))
        return (w2 - w1) / (m2 - m1), w1, w2


# ----------------------------------------------------------------------------
# Host-side entry: shard full inputs over the 8 NeuronCores, run, gather.
# ----------------------------------------------------------------------------
B, S, D, H = 4, 2048, 1024, 16
SQ = S // 2
NCORES = 8
QK8 = False

_runner_cache = []


def _get_runner():
    if not _runner_cache:
        nc = build_mha(S, D, H, SQ, attn_dt=mybir.dt.float16, qk8=QK8)
        split_ctrl_multiwaits(nc)
        _runner_cache.append(SpmdRunner(nc, NCORES))
    return _runner_cache[0]


def _make_in_maps(q, k, v, mask, Wq, Wk, Wv, Wo):
    f16 = np.float16
    wq16 = np.ascontiguousarray(Wq, dtype=f16)
    wk16 = np.ascontiguousarray(Wk, dtype=f16)
    wv16 = np.ascontiguousarray(Wv, dtype=f16)
    wo16 = np.ascontiguousarray(Wo, dtype=f16)
    in_maps = []
    for c in range(NCORES):
        b, qh = c // 2, c % 2
        in_maps.append({
            "xqT": np.ascontiguousarray(q[b, qh * SQ:(qh + 1) * SQ].T, dtype=f16),
            "ktd": np.ascontiguousarray(k[b].T, dtype=f16),
            "vtd": np.ascontiguousarray(v[b].T, dtype=f16),
            "mkT": np.ascontiguousarray(mask[b, qh * SQ:(qh + 1) * SQ].T, dtype=f16),
            "wq": wq16, "wk": wk16, "wv": wv16, "wo": wo16,
        })
    return in_maps


def kernel(q, k, v, mask, Wq, Wk, Wv, Wo):
    q = np.asarray(q, np.float32)
    k = np.asarray(k, np.float32)
    v = np.asarray(v, np.float32)
    mask = np.asarray(mask, np.int32)
    Wq, Wk, Wv, Wo = (np.asarray(a, np.float32) for a in (Wq, Wk, Wv, Wo))
    r = _get_runner()
    r.stage(_make_in_maps(q, k, v, mask, Wq, Wk, Wv, Wo))
    res = r.results(r.run())
    out = np.empty((B, S, D), np.float32)
    for c in range(NCORES):
        b, qh = c // 2, c % 2
        out[b, qh * SQ:(qh + 1) * SQ] = res[c]["out"]
    return out
